# revision 1
# baseline (speedup 1.0000x reference)
"""BagOfWords Trainium2 kernel (bf16 pipeline).

Reference computation (per batch b):
    emb    = emb_table[context]                      # (T, D) gather
    logits = emb @ W.T + b                           # (T, V)
    out[t] = (sum_{s<=t} (s+1) * logits[s]) / den[t] # weighted causal cum-avg
    den[t] = (t+1)(t+2)/2

Key identity: the weighted cumsum commutes with the GEMM:
    out[t, v] = (num[t] @ W[v]) / den[t] + b[v]
    num[t, d] = sum_{s<=t} (s+1) * emb[s, d]
so the O(T*V) cumsum collapses onto the tiny (T, D) embedding side.
On device, per 128-token chunk (PE / ACT):
    psum[d, t] = sum_s emb[s, d] * UTW_c[s, t]      # one matmul per d-chunk
    NT[d, t]   = psum[d, t] + NT_prev[d, last]      # ACT copy w/ bias
with UTW_c[s, t] = (c*128+s+1) * [s <= t] -- the position weights folded
into the per-chunk upper-triangular constant, so there is no separate
scale pass.  The carry between chunks is the previous chunk's LAST COLUMN
of NT, consumed as the per-partition bias of the ACT PSUM->SBUF copy.
NT comes out pre-transposed (d on partitions) = exactly the lhsT layout the
big GEMM wants.  Then out = (NT.T @ W.T) * (1/den[t]) with the normalization
applied as a per-partition scalar in the PSUM->SBUF eviction, split across
ACT and DVE (2 tiles each per 4-tile store group), and streamed to HBM.

Everything that touches HBM is bf16 (table, weights, output, consts);
PSUM accumulation stays fp32.  Measured end-to-end rel err ~3e-3 vs the
fp32 reference (gate is 2e-2).  bf16 halves DMA traffic (42 -> 21 MB/core)
and moves the bottleneck to the PE: the big GEMM is 192K moving columns
= 80us at 1 col/cycle/2.4GHz, which bf16 sustains (fp32 would be 4x).

A small fp32 shadow column (carry_sb) tracks each chunk's last NT column:
it feeds the next chunk's ACT bias (which must be fp32 on hardware) and
keeps the carry chain exact instead of re-rounding through bf16.

Sharding (8 cores): 4-way over B x 2-way over V.  Each core gathers 2
batches (2048 rows) but holds only half of W -- the DMA-optimal split.

Raw Bass with manual semaphores (one wait per instruction): the walrus build
in this container rejects instructions carrying multiple sem waits.

DMA semaphore discipline: a DMA's 16 per-SDMA-engine sem increments interleave
arbitrarily with other in-flight DMAs on the same semaphore, so every
concurrently-outstanding DMA group gets its own semaphore, waited to exactly
16 per iteration.

reps>1 repeats the whole pipeline inside one NEFF (used only for timing).
Iterations re-gather from the table so every rep computes identical values;
cross-iteration WAR hazards get explicit waits.
"""

import functools
import os
from contextlib import ExitStack

import numpy as np

import concourse.bass as bass
from concourse import mybir
from concourse.bass_utils import run_bass_kernel_spmd

B, T, V, D = 8, 1024, 8000, 384
P = 128
NCORE = 8
NCHUNK = T // P                 # 8 token chunks per batch
KD = D // P                     # 3 contraction chunks
NV = 500                        # vocab tile (one fp32 PSUM bank)
VGRP = 4                        # vocab tiles per store group
NSTAGE = 4                      # output staging buffers
GEMM_BANKS = 6
F32 = mybir.dt.float32
BF16 = mybir.dt.bfloat16

NVG = int(os.environ.get("BOW_NVG", "2"))   # vocab groups (1 or 2)
WARM = int(os.environ.get("BOW_WARM", "0"))   # PE ramp warm-up matmuls
NB = NVG                        # batches per core (B=8, 8 cores)
V_CORE = V // NVG               # vocab columns per core
BT = NB * T                     # tokens per core
NCHT = NB * NCHUNK              # token chunks per core
NTV = V_CORE // NV              # vocab tiles per core
NGRP = NTV // VGRP              # store column groups
GCOLS = VGRP * NV               # columns per weight/store group

# bf16 const-block column layout (single DMA, single sem)
C_UTW = 0                       # [128, 8*128] per-chunk (s+1)-weighted tril^T
C_DENROW = C_UTW + NCHUNK * P   # row 0, [1, 1024] den[t] (bias path only)
C_BIAS = C_DENROW + T           # row 0, [1, V_CORE] (only when has_bias)
CW16_NOBIAS = C_UTW + NCHUNK * P
CW16_BIAS = C_BIAS + V_CORE
# fp32 const block: idenc only
CW32 = NCHUNK                   # [128, 8] 1/den[c*128+p] column layout

# one single-chunk gather per 128 tokens: multi-chunk offset APs scramble the
# destination layout on real hardware (descgen ucode disagrees with the
# interpreter) and can write out of bounds, and dma_gather does not compile
# in this container's walrus -- so the SWDGE preps stay serial on Pool.
GATHER_GROUPS = [1] * NCHT
assert sum(GATHER_GROUPS) == NCHT
_GSTART = [sum(GATHER_GROUPS[:i]) for i in range(len(GATHER_GROUPS))]
_GROUP_OF = [max(i for i, s in enumerate(_GSTART) if s <= cc)
             for cc in range(NCHT)]

# per-iteration semaphore increments
CT_IT = NCHT * KD               # ctdone / ctsb
GM_IT = NCHT * NTV              # pegemm tiles
GR_IT = GM_IT // VGRP           # store groups
AE_IT = GM_IT // 2              # ACT evictions (tiles nin 0,1)
DE_IT = GM_IT // 2              # DVE evictions (tiles nin 2,3)


def _evict_count(a):
    """Engine-local (engine, 1-based count) for absolute gemm tile a.
    The very first block's four tiles all evict on DVE (ACT is busy with
    the startup carry/NT chain), shifting later counts by +-2."""
    if a < VGRP:
        return "d", a + 1
    r = a % VGRP
    if r < 2:
        return "a", 2 * (a // VGRP) + r - 1
    return "d", 2 * (a // VGRP) + r + 1


def _ctsb_count(it, nchunks):
    """DVE ctsb increments once `nchunks` chunks of iter `it` are copied."""
    return KD * (it * NCHT + nchunks)


def _build(has_bias: bool, reps: int = 1, dbg: bool = False):
    nc = bass.Bass("TRN2", target_bir_lowering=False, debug=False)

    CW16 = CW16_BIAS if has_bias else CW16_NOBIAS

    idx_d = nc.dram_tensor("idx", [P, NCHT], mybir.dt.int32, kind="ExternalInput")
    table_d = nc.dram_tensor("table", [V, D], BF16, kind="ExternalInput")
    wt_d = nc.dram_tensor("wt", [D, V_CORE], BF16, kind="ExternalInput")
    cst16_d = nc.dram_tensor("cst16", [P, CW16], BF16, kind="ExternalInput")
    cst32_d = nc.dram_tensor("cst32", [P, CW32], F32, kind="ExternalInput")
    out_d = nc.dram_tensor("out", [BT, V_CORE], BF16, kind="ExternalOutput")
    if dbg:
        demb_d = nc.dram_tensor("demb", [P, NCHT * D], BF16, kind="ExternalOutput")
        dct_d = [nc.dram_tensor(f"dct{k}", [P, BT], BF16, kind="ExternalOutput")
                 for k in range(KD)]
        dcarry_d = nc.dram_tensor("dcarry", [P, KD * NCHT], F32, kind="ExternalOutput")

    with ExitStack() as ctx:
        e = ctx.enter_context
        # SBUF
        idx_sb = e(nc.sbuf_tensor("idx_sb", [P, NCHT], mybir.dt.int32))
        cst16 = e(nc.sbuf_tensor("cst16_sb", [P, CW16], BF16))
        cst32 = e(nc.sbuf_tensor("cst32_sb", [P, CW32], F32))
        emb_sb = e(nc.sbuf_tensor("emb_sb", [P, NCHT * D], BF16))
        ct_sb = [e(nc.sbuf_tensor(f"ct{k}", [P, BT], BF16)) for k in range(KD)]
        # fp32 shadow of each chunk's last NT column: the ACT bias operand
        # must be fp32 on hardware, and fp32 carry also kills the bf16
        # carry-rounding accumulation
        carry_sb = e(nc.sbuf_tensor("carry_sb", [P, KD * NCHT], F32))
        wt_sb = [e(nc.sbuf_tensor(f"wt{k}", [P, V_CORE], BF16)) for k in range(KD)]
        ostg = [e(nc.sbuf_tensor(f"ostg{q}", [P, VGRP * NV], BF16)) for q in range(NSTAGE)]
        # PSUM (8 banks: 6 gemm + 2 prefix).  One prefix bank holds a whole
        # chunk (KD*128 = 384 fp32 cols), so the 3 per-chunk matmuls never
        # WAR against their own chunk's ACT copies -- only chunk cc-2's.
        gps = [e(nc.psum_tensor(f"gps{i}", [P, NV], F32)) for i in range(GEMM_BANKS)]
        ctps = [e(nc.psum_tensor(f"ctps{i}", [P, KD * P], F32)) for i in range(2)]
        # sems -- one per concurrently-outstanding DMA group
        csem16 = e(nc.semaphore("csem16"))
        csem32 = e(nc.semaphore("csem32"))
        wsem = [[e(nc.semaphore(f"wsem{k}_{g}")) for g in range(NGRP)] for k in range(KD)]
        wsemh = [[e(nc.semaphore(f"wsemh{k}_{g}")) for g in range(NGRP)] for k in range(KD)]
        # group-0 second-half weights split into per-tile 500-col pieces so
        # the first block's nin2/nin3 k-slices land as early as possible
        wq2 = [e(nc.semaphore(f"wq2_{k}")) for k in range(KD)]
        wq3 = [e(nc.semaphore(f"wq3_{k}")) for k in range(KD)]
        gidx = e(nc.semaphore("gidx"))
        gsem = [e(nc.semaphore(f"gsem{gg}")) for gg in range(len(GATHER_GROUPS))]
        osem = [e(nc.semaphore(f"osem{q}")) for q in range(NSTAGE)]
        # engine-progress sems (single-inc, exactly ordered)
        ctdone = e(nc.semaphore("ctdone"))  # prefix psum tiles done
        ctsb = e(nc.semaphore("ctsb"))      # NT psum->sbuf copies
        pegemm = e(nc.semaphore("pegemm"))  # gemm psum tiles done
        asem = e(nc.semaphore("asem"))      # ACT evictions
        dsem = e(nc.semaphore("dsem"))      # DVE evictions
        carrysem = e(nc.semaphore("carrysem"))  # ACT fp32 carry columns
        blk = e(nc.Block())

        def emb_cc(cc):
            return emb_sb[:, cc * D:(cc + 1) * D]

        utw_ap = lambda c: cst16[:, C_UTW + c * P:C_UTW + (c + 1) * P]
        idenc_ap = lambda c: cst32[:, c:c + 1]
        denrow_ap = lambda c: cst16[0:1, C_DENROW + c * P:C_DENROW + (c + 1) * P]
        bias_ap = lambda n: cst16[0:1, C_BIAS + n * NV:C_BIAS + (n + 1) * NV]

        @blk.sync
        def _(sync):
            # idx first: the gather chain (idx -> SWDGE prep -> gather 0) is
            # the critical path to the first prefix matmul
            sync.dma_start(idx_sb[:], idx_d[:]).then_inc(gidx, 16)
            sync.dma_start(cst16[:], cst16_d[:]).then_inc(csem16, 16)
            sync.dma_start(cst32[:], cst32_d[:]).then_inc(csem32, 16)
            # group-0 weights k-major in three bands (first half, then two
            # 500-col quarters) so each of the first block's tiles unblocks
            # as early as possible; group-1 halves held until the gathers
            # are through the DMA queue
            g0_bands = [(0, GCOLS // 2, wsemh), (GCOLS // 2, GCOLS // 2 + NV, wq2),
                        (GCOLS // 2 + NV, GCOLS, wq3)]
            for c0, c1, sems in g0_bands:
                for k in range(KD):
                    dma = sync.dma_start(wt_sb[k][:, c0:c1],
                                         wt_d[k * P:(k + 1) * P, c0:c1])
                    dma.then_inc(sems[k][0] if sems is wsemh else sems[k], 16)
            for h in range(2):
                for k in range(KD):
                    sync.wait_ge(gsem[min(h * 3 + k + 1,
                                          len(GATHER_GROUPS) - 1)], 16)
                    cols = slice(GCOLS + h * GCOLS // 2,
                                 GCOLS + (h + 1) * GCOLS // 2)
                    dma = sync.dma_start(wt_sb[k][:, cols],
                                         wt_d[k * P:(k + 1) * P, cols])
                    if h == 1:
                        dma.then_inc(wsem[k][1], 16)
                    else:
                        dma.then_inc(wsemh[k][1], 16)
            # output stores (SP's DGE ring is free once the weights are out)
            for it in range(reps):
                for g in range(NGRP):
                    for mc in range(NCHT):
                        gi = it * GR_IT + g * NCHT + mc
                        last_grp = gi == reps * GR_IT - 1
                        if not last_grp:
                            if gi > 0:
                                sync.wait_ge(asem, 2 * (gi + 1) - 2)
                            sync.wait_ge(dsem, 2 * (gi + 1) + (2 if gi > 0 else 2))
                            sync.dma_start(
                                out_d[mc * P:(mc + 1) * P,
                                      g * GCOLS:(g + 1) * GCOLS],
                                ostg[gi % NSTAGE][:],
                            ).then_inc(osem[gi % NSTAGE], 16)
                        else:
                            # final group: per-tile stores so each tile
                            # streams out as soon as its eviction lands
                            for nin in range(VGRP):
                                eng, cnt = _evict_count(gi * VGRP + nin)
                                sync.wait_ge(asem if eng == "a" else dsem, cnt)
                                sync.dma_start(
                                    out_d[mc * P:(mc + 1) * P,
                                          g * GCOLS + nin * NV:
                                          g * GCOLS + (nin + 1) * NV],
                                    ostg[gi % NSTAGE][:, nin * NV:(nin + 1) * NV],
                                ).then_inc(osem[gi % NSTAGE], 16)
            for q in range(NSTAGE):
                ngrp_q = (reps * GR_IT - q + NSTAGE - 1) // NSTAGE
                if (reps * GR_IT - 1) % NSTAGE == q:
                    ngrp_q += VGRP - 1   # final group incs osem per tile
                sync.wait_ge(osem[q], 16 * ngrp_q)
            if dbg:
                dbgsem = nc.semaphore("dbgsem").__enter__()
                sync.dma_start(demb_d[:], emb_sb[:]).then_inc(dbgsem, 16)
                for k in range(KD):
                    sync.dma_start(dct_d[k][:], ct_sb[k][:]).then_inc(dbgsem, 16)
                sync.dma_start(dcarry_d[:], carry_sb[:]).then_inc(dbgsem, 16)
                sync.wait_ge(dbgsem, 16 * (KD + 2))

        @blk.gpsimd
        def _(gpsimd):
            gpsimd.wait_ge(gidx, 16)
            for it in range(reps):
                for gg, ng in enumerate(GATHER_GROUPS):
                    c0 = _GSTART[gg]
                    if it > 0:
                        # WAR: PE must be done reading these chunks of iter it-1
                        gpsimd.wait_ge(ctdone, (it - 1) * CT_IT + (c0 + ng) * KD)
                    gpsimd.indirect_dma_start(
                        out=emb_sb[:, c0 * D:(c0 + ng) * D],
                        out_offset=None,
                        in_=table_d[:],
                        in_offset=bass.IndirectOffsetOnAxis(
                            ap=idx_sb[:, c0:c0 + ng], axis=0),
                    ).then_inc(gsem[gg], 16)

        def act_evict(scalar, a, mc):
            gi = a // VGRP
            nin = a % VGRP              # 0 or 1 on ACT
            if nin == 0 and gi >= NSTAGE:
                scalar.wait_ge(osem[gi % NSTAGE], 16 * (gi // NSTAGE))
            scalar.wait_ge(pegemm, a + 1)
            scalar.mul(ostg[gi % NSTAGE][:, nin * NV:(nin + 1) * NV],
                       gps[a % GEMM_BANKS][:],
                       idenc_ap(mc % NCHUNK)).then_inc(asem, 1)

        @blk.scalar
        def _(scalar):
            scalar.wait_ge(csem32, 16)
            for it in range(reps):
                def carr(cc):
                    # fp32 carry columns (psum's last column + prev carry),
                    # then the bf16 NT copy using that fp32 carry as bias.
                    jc = it * NCHT + cc
                    for k in range(KD):
                        j = it * CT_IT + cc * KD + k
                        scalar.wait_ge(ctdone, j + 1)
                        if it > 0 and cc == 0 and k == 0:
                            # WAR: gemm of iter it-1 must be done reading ct_sb
                            scalar.wait_ge(pegemm, it * GM_IT)
                        dst = carry_sb[:, cc * KD + k:cc * KD + k + 1]
                        src = ctps[jc % 2][:, k * P + P - 1:k * P + P]
                        ntdst = ct_sb[k][:, cc * P:(cc + 1) * P]
                        ntsrc = ctps[jc % 2][:, k * P:(k + 1) * P]
                        # NT first (the gemm waits on it), carry second
                        if cc % NCHUNK == 0:
                            scalar.copy(ntdst, ntsrc).then_inc(ctsb, 1)
                            scalar.copy(dst, src).then_inc(carrysem, 1)
                        else:
                            prev = carry_sb[:, (cc - 1) * KD + k:(cc - 1) * KD + k + 1]
                            scalar.add(ntdst, ntsrc, prev).then_inc(ctsb, 1)
                            scalar.add(dst, src, prev).then_inc(carrysem, 1)

                # carry columns lead the gemm by one chunk; ACT evictions for
                # the g0 sweep trail the block that produced them
                carr(0)
                for mc in range(NCHT):
                    if mc + 1 < NCHT:
                        carr(mc + 1)
                    a0 = it * GM_IT + mc * VGRP
                    if it == 0 and mc == 0:
                        continue   # first block's evictions run on DVE
                    act_evict(scalar, a0, mc)
                    act_evict(scalar, a0 + 1, mc)
                for g in range(1, NGRP):
                    for mc in range(NCHT):
                        a0 = it * GM_IT + (g * NCHT + mc) * VGRP
                        act_evict(scalar, a0, mc)
                        act_evict(scalar, a0 + 1, mc)

        @blk.tensor
        def _(tensor):
            # PE p-state warm-up: dummy matmuls on (uninitialized) SBUF while
            # the idx/gather DMA chain is in flight, so the real pipeline
            # starts at the full 2.4 GHz clock.  The garbage results land in a
            # ctps bank that the first real prefix overwrites (start=True),
            # and WAW on ctps is same-engine-ordered.
            for w in range(WARM):
                tensor.matmul(ctps[w % 2][:, 0:P], lhsT=emb_sb[:, 0:P],
                              rhs=emb_sb[:, 0:P], start=True, stop=True)
            tensor.wait_ge(csem16, 16)
            for it in range(reps):
                def prefix(cc):
                    # one matmul per (chunk, d-slice) into one bank; position
                    # weights are in UTW; the carry is applied by the ACT copy
                    tensor.wait_ge(gsem[_GROUP_OF[cc]], 16 * (it + 1))
                    jc = it * NCHT + cc
                    if jc >= 2:
                        # WAR on ctps bank: chunk jc-2's NT copies (DVE) and
                        # carry columns (ACT) must both be done
                        tensor.wait_ge(ctsb, KD * (jc - 1))
                        tensor.wait_ge(carrysem, KD * (jc - 1))
                    for k in range(KD):
                        tensor.matmul(
                            ctps[jc % 2][:, k * P:(k + 1) * P],
                            lhsT=emb_sb[:, cc * D + k * P: cc * D + (k + 1) * P],
                            rhs=utw_ap(cc % NCHUNK),
                            start=True, stop=True).then_inc(ctdone, 1)

                def gemm_block(g, mc, pf=None):
                    split0 = it == 0 and mc == 0   # chunk-0 NT copies split
                    if g == 0 and not split0:
                        tensor.wait_ge(ctsb, _ctsb_count(it, mc + 1))
                    for nin in range(VGRP):
                        n = g * VGRP + nin
                        a = it * GM_IT + (g * NCHT + mc) * VGRP + nin
                        if a >= GEMM_BANKS:
                            eng, cnt = _evict_count(a - GEMM_BANKS)
                            tensor.wait_ge(asem if eng == "a" else dsem, cnt)
                        for k in range(KD):
                            if split0 and g == 0 and nin == 0:
                                # per-k NT waits: start on each k-slice the
                                # moment its DVE copy lands
                                tensor.wait_ge(ctsb, k + 1)
                            if it == 0 and mc == 0 and nin == 0:
                                tensor.wait_ge(wsemh[k][g], 16)
                            if it == 0 and mc == 0 and nin == 2:
                                tensor.wait_ge(wq2[k] if g == 0 else wsem[k][g], 16)
                            if it == 0 and mc == 0 and nin == 3 and g == 0:
                                tensor.wait_ge(wq3[k], 16)
                            last = (k == KD - 1) and not has_bias
                            mm = tensor.matmul(
                                gps[a % GEMM_BANKS][:],
                                lhsT=ct_sb[k][:, mc * P:(mc + 1) * P],
                                rhs=wt_sb[k][:, n * NV:(n + 1) * NV],
                                start=(k == 0), stop=last)
                        if has_bias:
                            mm = tensor.matmul(
                                gps[a % GEMM_BANKS][:],
                                lhsT=denrow_ap(mc % NCHUNK),
                                rhs=bias_ap(n),
                                start=False, stop=True)
                        mm.then_inc(pegemm, 1)
                        if nin == 0 and pf is not None:
                            prefix(pf)

                # uniform sweep: prefix(mc+1) is interleaved after the first
                # tile of block (0, mc) so its NT copies overlap the block
                prefix(0)
                for mc in range(NCHT):
                    gemm_block(0, mc, pf=mc + 1 if mc + 1 < NCHT else None)
                for g in range(1, NGRP):
                    for mc in range(NCHT):
                        gemm_block(g, mc)

        @blk.vector
        def _(vector):
            vector.wait_ge(csem32, 16)

            def evict(it, g, mc, nin):
                a = it * GM_IT + (g * NCHT + mc) * VGRP + nin
                gi = a // VGRP
                if nin == 2 and gi >= NSTAGE:
                    vector.wait_ge(osem[gi % NSTAGE], 16 * (gi // NSTAGE))
                vector.wait_ge(pegemm, a + 1)
                vector.tensor_scalar_mul(
                    ostg[gi % NSTAGE][:, nin * NV:(nin + 1) * NV],
                    gps[a % GEMM_BANKS][:],
                    idenc_ap(mc % NCHUNK)).then_inc(dsem, 1)

            for it in range(reps):
                for g in range(NGRP):
                    for mc in range(NCHT):
                        if it == 0 and g == 0 and mc == 0:
                            evict(it, g, mc, 0)
                            evict(it, g, mc, 1)
                        evict(it, g, mc, 2)
                        evict(it, g, mc, 3)

    return nc


@functools.lru_cache(maxsize=None)
def _get_program(has_bias: bool, reps: int = 1, dbg: bool = False):
    return _build(has_bias, reps, dbg)


@functools.lru_cache(maxsize=None)
def _host_consts(has_bias: bool):
    import ml_dtypes
    CW16 = CW16_BIAS if has_bias else CW16_NOBIAS
    c16 = np.zeros((P, CW16), dtype=ml_dtypes.bfloat16)
    t = np.arange(T, dtype=np.float64)
    den = (t + 1.0) * (t + 2.0) / 2.0
    s = np.arange(P)
    tril_t = (s[:, None] <= s[None, :]).astype(np.float32)  # [s, t] s<=t
    for c in range(NCHUNK):
        posw = (np.arange(c * P, (c + 1) * P, dtype=np.float32) + 1.0)
        c16[:, C_UTW + c * P:C_UTW + (c + 1) * P] = (
            posw[:, None] * tril_t).astype(ml_dtypes.bfloat16)
    if has_bias:
        c16[0, C_DENROW:C_DENROW + T] = den.astype(ml_dtypes.bfloat16)
    c32 = np.zeros((P, CW32), dtype=np.float32)
    c32[:, :] = (1.0 / den).astype(np.float32).reshape(NCHUNK, P).T
    return c16, c32


def make_in_maps(context, emb_table, W, b):
    import ml_dtypes
    context = np.asarray(context)
    emb_table = np.asarray(emb_table, dtype=np.float32)
    W = np.asarray(W, dtype=np.float32)
    b = np.asarray(b, dtype=np.float32)
    has_bias = bool(np.any(b))

    table16 = np.ascontiguousarray(emb_table.astype(ml_dtypes.bfloat16))
    wt_full = np.ascontiguousarray(W.T.astype(ml_dtypes.bfloat16))  # (D, V)
    c16_0, c32 = _host_consts(has_bias)

    in_maps = []
    for ci in range(NCORE):
        vg, bg = ci % NVG, ci // NVG
        idx = np.concatenate(
            [context[bg * NB + bt].reshape(NCHUNK, P).T for bt in range(NB)],
            axis=1).astype(np.int32)           # [p, cc]
        wt = np.ascontiguousarray(wt_full[:, vg * V_CORE:(vg + 1) * V_CORE])
        c16 = c16_0
        if has_bias:
            c16 = c16_0.copy()
            c16[0, C_BIAS:C_BIAS + V_CORE] = \
                b[vg * V_CORE:(vg + 1) * V_CORE].astype(ml_dtypes.bfloat16)
        in_maps.append({"idx": np.ascontiguousarray(idx), "table": table16,
                        "wt": wt, "cst16": c16, "cst32": c32})
    return in_maps, has_bias


def kernel(context, emb_table, W, b):
    in_maps, has_bias = make_in_maps(context, emb_table, W, b)
    nc = _get_program(has_bias)
    res = None
    for attempt in range(3):
        try:
            res = run_bass_kernel_spmd(nc, in_maps, list(range(NCORE)))
            break
        except Exception:
            # the axon-tunneled device occasionally reports a transient
            # NRT_EXEC_UNIT_UNRECOVERABLE / INTERNAL error; back off and retry
            if attempt == 2:
                raise
            import time
            time.sleep(10.0 * (attempt + 1))
    out = np.empty((B, T, V), dtype=np.float32)
    for ci in range(NCORE):
        vg, bg = ci % NVG, ci // NVG
        o = np.asarray(res.results[ci]["out"]).astype(np.float32)
        for bt in range(NB):
            out[bg * NB + bt, :, vg * V_CORE:(vg + 1) * V_CORE] = \
                o[bt * T:(bt + 1) * T]
    return out



# revision 33
# speedup vs baseline: 1.4394x; 1.4394x over previous
"""BagOfWords Trainium2 kernel (fp8 DoubleRow pipeline).

Reference computation (per batch b):
    emb    = emb_table[context]                      # (T, D) gather
    logits = emb @ W.T + b                           # (T, V)
    out[t] = (sum_{s<=t} (s+1) * logits[s]) / den[t] # weighted causal cum-avg
    den[t] = (t+1)(t+2)/2

Key identity: the weighted cumsum commutes with the GEMM:
    out[t, v] = (num[t] @ W[v]) / den[t] + b[v]
    num[t, d] = sum_{s<=t} (s+1) * emb[s, d]
so the O(T*V) cumsum collapses onto the tiny (T, D) embedding side.
Per 128-token chunk (PE / DVE):
    psum[d, t] = sum_s emb[s, d] * UTW_c[s, t]       # prefix matmul per d-chunk
    NT[d, t]   = psum[d, t] + carry_prev[d]          # DVE copy w/ carry scalar
with the carry chain kept exact in fp32 (carry_sb) via paired DVE
tensor_tensor updates of the psum's last columns.

fp8 acceleration: the big GEMM out = NT.T @ W.T runs in fp8e4m3 with
MatmulPerfMode.DoubleRow -- K=256 per matmul at 0.5 cycles/moving-column,
2x the bf16 rate.  D=384 is zero-padded to 512 (2 DoubleRow matmuls/tile;
the 4th k-slice of both NT and W is zeroed).  Precision: the output's
global max lives at EARLY tokens (den[t] ~ t^2 makes late outputs tiny),
so chunk 0 of each batch stays on the bf16 path (bf16 NT, bf16 W, bf16
output) while chunks 1..7 use fp8 NT (per-chunk pow2 scale 2^-E_NT[c]),
fp8 W (2^EW), and fp8 *output* (per-chunk 2^S_OUT[c] folded into the
eviction constant, dequantized on host).  Measured end-to-end rel err
~5e-3 vs the fp32 reference (gate is 2e-2).

fp8 output also cuts the dominant HBM store traffic 2x (16.4 -> 9.2
MB/core total DMA ~16 MB ~ 44 us at 360 GB/s), and DoubleRow cuts PE time
80 -> 36 us.  PSUM->SBUF evictions (per-partition 1/den scale + dtype
convert) are the third resource: GPSIMD cannot touch PSUM, so they are
split between ACT and DVE in 2-tile (1000-column) pairs over 2-bank PSUM
tensors to amortize init overhead; DVE additionally owns the NT copies
((psum + carry)*2^-e in one two-scalar tensor_scalar op) and the paired
carry updates.

Sharding (8 cores): 4-way over B x 2-way over V.  Each core gathers 2
batches (2048 rows) and holds half of W (bf16 + fp8 copies).

Raw Bass with manual semaphores (one wait per instruction): the walrus build
in this container rejects instructions carrying multiple sem waits.

DMA semaphore discipline: a DMA's 16 per-SDMA-engine sem increments interleave
arbitrarily with other in-flight DMAs on the same semaphore, so every
concurrently-outstanding DMA group gets its own semaphore, waited to exactly
16 per iteration.

reps>1 repeats the whole pipeline inside one NEFF (used only for timing).
"""

import functools
import os
from contextlib import ExitStack

import numpy as np

import concourse.bass as bass
from concourse import mybir
from concourse.bass_utils import run_bass_kernel_spmd

B, T, V, D = 8, 1024, 8000, 384
P = 128
NCORE = 8
NCHUNK = T // P                 # 8 token chunks per batch
KD = D // P                     # 3 real contraction chunks
NV = 500                        # vocab tile (one fp32 PSUM bank half)
VGRP = 4                        # vocab tiles per store group
F32 = mybir.dt.float32
BF16 = mybir.dt.bfloat16
F8 = mybir.dt.float8e4
DR = mybir.MatmulPerfMode.DoubleRow
Alu = mybir.AluOpType

NVG = 2                         # vocab groups (cores split 4B x 2V)
WARM = int(os.environ.get("BOW_WARM", "0"))
NB = NVG                        # batches per core
V_CORE = V // NVG               # 4000 vocab columns per core
BT = NB * T                     # 2048 tokens per core
NCHT = NB * NCHUNK              # 16 token chunks per core
NTV = V_CORE // NV              # 8 vocab tiles per core
NGRP = NTV // VGRP              # 2 store column groups
GCOLS = VGRP * NV               # 2000 columns per weight/store group
NBLK = NGRP * NCHT              # 32 gemm blocks per iteration
GM_IT = NBLK * VGRP             # gemm tiles per iteration
CT_IT = NCHT * KD               # NT copies per iteration
NPAIR = GM_IT // 2              # eviction pairs per iteration (64)
NST8 = 8                        # fp8 staging buffers
NST16 = 3                       # bf16 staging buffers

# fp8 scale exponents (host-validated: global rel err ~5.4e-3)
EW = 6                                       # W8 = W * 2^EW
E_NT = [0, 6, 6, 7, 8, 8, 8, 9]              # NT8 = NT * 2^-E_NT[c]
S_OUT = [0, 11, 11, 11, 11, 12, 12, 12]      # out8 = out * 2^S_OUT[c]

# one single-chunk gather per 128 tokens (multi-chunk offset APs scramble
# the destination layout on real hardware)
GATHER_GROUPS = [1] * NCHT

# gemm block sweep order (see block_seq): fp8 chunks first, chunk-0s after
# their batch's fp8 run has started; invariant CHUNK_ORDER[j] <= j+1
CHUNK_ORDER = [1, 2, 3, 4, 5, 6, 7, 0, 9, 8, 10, 11, 12, 13, 14, 15]

# --- eviction pair -> engine assignment ---------------------------------
# Blocks sweep g-INNER ((0,mc),(1,mc),(0,mc+1),...) so each chunk's NT-copy
# work spreads over two block periods.  Block bi has pairs (2bi, 2bi+1).
# DVE owns the (fused) NT copies + carry chain (~0.79us/chunk), so ACT
# takes 5 of every 8 pairs: per 2 chunks ACT 5x1.02 = 5.09us vs DVE
# 3x1.17 + 2x0.79 = 5.08us.

_PAT = ["a", "d", "a", "a", "d", "a", "a", "d"]


def _pair_engine(q):
    return _PAT[q % len(_PAT)]

_A_IT = sum(1 for x in range(NPAIR) if _pair_engine(x) == "a")
_D_IT = NPAIR - _A_IT


def _pair_count(q):
    """1-based per-engine count of pair q among pairs of its engine."""
    e = _pair_engine(q)
    it, qq = divmod(q, NPAIR)
    base = (_A_IT if e == "a" else _D_IT) * it
    return base + sum(1 for x in range(qq + 1) if _pair_engine(x) == e)


def _block_evict_counts(it, bi):
    """Cumulative (asem, dsem) counts once block bi's pairs are evicted."""
    a = sum(1 for x in range(2 * bi + 2) if _pair_engine(x) == "a") + _A_IT * it
    d = sum(1 for x in range(2 * bi + 2) if _pair_engine(x) == "d") + _D_IT * it
    return a, d


def _build(has_bias: bool, reps: int = 1, dbg: bool = False):
    nc = bass.Bass("TRN2", target_bir_lowering=False, debug=False)

    idx_d = nc.dram_tensor("idx", [P, NCHT], mybir.dt.int32, kind="ExternalInput")
    table_d = nc.dram_tensor("table", [V, D], BF16, kind="ExternalInput")
    wt_d = nc.dram_tensor("wt", [D, V_CORE], BF16, kind="ExternalInput")
    wt8_d = nc.dram_tensor("wt8", [P, NGRP * KD * GCOLS], F8, kind="ExternalInput")
    cst16_d = nc.dram_tensor("cst16", [P, NCHUNK * P], BF16, kind="ExternalInput")
    cst32_d = nc.dram_tensor("cst32", [P, NCHUNK], F32, kind="ExternalInput")
    out16_d = nc.dram_tensor("out16", [NB * P, V_CORE], BF16, kind="ExternalOutput")
    out8_d = nc.dram_tensor("out8", [NB * (NCHUNK - 1) * P, V_CORE], F8,
                            kind="ExternalOutput")

    with ExitStack() as ctx:
        e = ctx.enter_context
        # SBUF
        idx_sb = e(nc.sbuf_tensor("idx_sb", [P, NCHT], mybir.dt.int32))
        cst16 = e(nc.sbuf_tensor("cst16_sb", [P, NCHUNK * P], BF16))
        cst32 = e(nc.sbuf_tensor("cst32_sb", [P, NCHUNK], F32))
        emb_sb = e(nc.sbuf_tensor("emb_sb", [P, NCHT * D], BF16))
        # bf16 NT: chunk 0 of each batch only
        ct16 = e(nc.sbuf_tensor("ct16", [P, KD, NB * P], BF16))
        # fp8 NT: [p, k-slice (4th zeroed), token]; chunk-0 columns unused
        ct8 = e(nc.sbuf_tensor("ct8", [P, 4, BT], F8))
        carry_sb = e(nc.sbuf_tensor("carry_sb", [P, KD * NCHT], F32))
        # pre-scaled carry columns (carry * 2^-e) for the fused NT copy
        carrysc_sb = e(nc.sbuf_tensor("carrysc_sb", [P, KD * NCHT], F32))
        wt_sb = [e(nc.sbuf_tensor(f"wt{k}", [P, V_CORE], BF16)) for k in range(KD)]
        wt8_sb = e(nc.sbuf_tensor("wt8_sb", [P, NGRP * KD, GCOLS], F8))
        ostg8 = [e(nc.sbuf_tensor(f"ostg8_{q}", [P, VGRP, NV], F8))
                 for q in range(NST8)]
        ostg16 = [e(nc.sbuf_tensor(f"ostg16_{q}", [P, VGRP, NV], BF16))
                  for q in range(NST16)]
        # PSUM: 3 x 2-bank gemm pair tensors + 2 x 1-bank prefix tensors
        gps = [e(nc.psum_tensor(f"gps{i}", [P, 2, 512], F32)) for i in range(3)]
        ctps = [e(nc.psum_tensor(f"ctps{i}", [P, KD * P], F32)) for i in range(2)]
        # sems
        gidx = e(nc.semaphore("gidx"))
        csem16 = e(nc.semaphore("csem16"))
        csem32 = e(nc.semaphore("csem32"))
        wsem16 = [e(nc.semaphore(f"wsem16_{g}")) for g in range(NGRP)]
        w8sem = [e(nc.semaphore(f"w8sem_{g}")) for g in range(NGRP)]
        k3sem = e(nc.semaphore("k3sem"))
        gsem = [e(nc.semaphore(f"gsem{gg}")) for gg in range(NCHT)]
        ctdone = e(nc.semaphore("ctdone"))      # prefix psum matmuls (PE)
        ctsbD = e(nc.semaphore("ctsbD"))        # fused NT copies (DVE)
        carrysem = e(nc.semaphore("carrysem"))  # carry updates (DVE)
        pegemm = e(nc.semaphore("pegemm"))      # gemm tiles (PE)
        asem = e(nc.semaphore("asem"))          # ACT pair evictions
        dsem = e(nc.semaphore("dsem"))          # DVE pair evictions
        osem8 = [e(nc.semaphore(f"osem8_{q}")) for q in range(NST8)]
        osem16 = [e(nc.semaphore(f"osem16_{q}")) for q in range(NST16)]
        blk = e(nc.Block())

        utw_ap = lambda c: cst16[:, c * P:(c + 1) * P]
        evc_ap = lambda c: cst32[:, c:c + 1]

        # block order = gemm order: g-INNER ((0,mc),(1,mc),(0,mc'),...) over
        # CHUNK_ORDER, which runs the fp8 chunks FIRST (their weights are
        # 1.5 MB vs bf16's 3 MB, so the gemm starts ~15us earlier) and slots
        # each batch's bf16 chunk-0 in once wt16 has landed.  Legal because
        # the NT chain (natural order 0..15) is decoupled from block order:
        # CHUNK_ORDER[j] <= j+1 for all j.
        def block_seq():
            for mc in CHUNK_ORDER:
                for g in range(NGRP):
                    yield g, mc

        def is16(mc):
            return mc % NCHUNK == 0

        # staging rotation indices, per iteration-local block index
        _stg = {}
        n8 = n16 = 0
        for _bi, (_g, _mc) in enumerate(block_seq()):
            if is16(_mc):
                _stg[_bi] = ("16", n16)
                n16 += 1
            else:
                _stg[_bi] = ("8", n8)
                n8 += 1
        N8_IT, N16_IT = n8, n16      # 28 fp8 / 4 bf16 blocks per iter

        def stg_buf(it, bi):
            kind, n = _stg[bi]
            if kind == "8":
                gi = it * N8_IT + n
                return ostg8[gi % NST8], osem8[gi % NST8], gi, NST8
            gi = it * N16_IT + n
            return ostg16[gi % NST16], osem16[gi % NST16], gi, NST16

        @blk.sync
        def _(sync):
            # idx first: the gather chain is the critical path
            sync.dma_start(idx_sb[:], idx_d[:]).then_inc(gidx, 16)
            sync.dma_start(cst32[:], cst32_d[:]).then_inc(csem32, 16)
            sync.dma_start(cst16[:], cst16_d[:]).then_inc(csem16, 16)
            # weights in need order (fp8 blocks run first), in ~0.7us bands
            # so the gather transfers interleave on the DMA bus
            for g in range(NGRP):
                for k in range(KD):
                    sync.dma_start(
                        wt8_sb[:, g * KD + k:g * KD + k + 1, :],
                        wt8_d[:, (g * KD + k) * GCOLS:(g * KD + k + 1) * GCOLS]
                    ).then_inc(w8sem[g], 16)
            for g in range(NGRP):
                for k in range(KD):
                    for h in range(2):
                        c0 = g * GCOLS + h * (GCOLS // 2)
                        sync.dma_start(
                            wt_sb[k][:, c0:c0 + GCOLS // 2],
                            wt_d[k * P:(k + 1) * P, c0:c0 + GCOLS // 2]
                        ).then_inc(wsem16[g], 16)
            # output stores in gemm-block order
            for it in range(reps):
                for bi, (g, mc) in enumerate(block_seq()):
                    buf, sem, gi, nst = stg_buf(it, bi)
                    ac, dc = _block_evict_counts(it, bi)
                    sync.wait_ge(asem, ac)
                    if dc > 0:
                        sync.wait_ge(dsem, dc)
                    if is16(mc):
                        bt = mc // NCHUNK
                        sync.dma_start(
                            out16_d[bt * P:(bt + 1) * P,
                                    g * GCOLS:(g + 1) * GCOLS],
                            buf[:, :, :]).then_inc(sem, 16)
                    else:
                        bt, c = divmod(mc, NCHUNK)
                        f = bt * (NCHUNK - 1) + (c - 1)
                        sync.dma_start(
                            out8_d[f * P:(f + 1) * P,
                                   g * GCOLS:(g + 1) * GCOLS],
                            buf[:, :, :]).then_inc(sem, 16)
            for q in range(NST8):
                tot = (reps * N8_IT - q + NST8 - 1) // NST8
                if tot > 0:
                    sync.wait_ge(osem8[q], 16 * tot)
            for q in range(NST16):
                tot = (reps * N16_IT - q + NST16 - 1) // NST16
                if tot > 0:
                    sync.wait_ge(osem16[q], 16 * tot)

        @blk.gpsimd
        def _(gpsimd):
            gpsimd.wait_ge(gidx, 16)
            for it in range(reps):
                for cc in range(NCHT):
                    if it > 0:
                        # WAR: PE must be done reading emb of iter it-1
                        gpsimd.wait_ge(ctdone, (it - 1) * CT_IT + (cc + 1) * KD)
                    gpsimd.indirect_dma_start(
                        out=emb_sb[:, cc * D:(cc + 1) * D],
                        out_offset=None,
                        in_=table_d[:],
                        in_offset=bass.IndirectOffsetOnAxis(
                            ap=idx_sb[:, cc:cc + 1], axis=0),
                    ).then_inc(gsem[cc], 16)

        @blk.tensor
        def _(tensor):
            for w in range(WARM):
                tensor.matmul(ctps[w % 2][:, 0:P], lhsT=emb_sb[:, 0:P],
                              rhs=emb_sb[:, 0:P], start=True, stop=True)
            tensor.wait_ge(csem16, 16)
            for it in range(reps):
                def prefix(cc):
                    tensor.wait_ge(gsem[cc], 16 * (it + 1))
                    jc = it * NCHT + cc
                    if jc >= 2:
                        # WAR on ctps bank: chunk jc-2's NT copy and carry
                        # (carry follows NT on DVE, so one wait covers both)
                        tensor.wait_ge(carrysem, jc - 1)
                    for k in range(KD):
                        tensor.matmul(
                            ctps[jc % 2][:, k * P:(k + 1) * P],
                            lhsT=emb_sb[:, cc * D + k * P: cc * D + (k + 1) * P],
                            rhs=utw_ap(cc % NCHUNK),
                            start=True, stop=True).then_inc(ctdone, 1)

                def gemm_block(g, mc, bi):
                    if g == 0:
                        tensor.wait_ge(ctsbD, it * NCHT + mc + 1)
                    for nin in range(VGRP):
                        a = it * GM_IT + bi * VGRP + nin
                        q = a // 2
                        if q >= 3:
                            qe = q - 3
                            sem = asem if _pair_engine(qe) == "a" else dsem
                            tensor.wait_ge(sem, _pair_count(qe))
                        gq, sl = gps[(a // 2) % 3], a % 2
                        if it == 0 and mc == (0 if is16(mc) else 1) and nin == 0:
                            if is16(mc):
                                tensor.wait_ge(wsem16[g], 96)
                            else:
                                tensor.wait_ge(w8sem[g], 48)
                                if g == 0:
                                    tensor.wait_ge(k3sem, 1)
                        if is16(mc):
                            bt = mc // NCHUNK
                            for k in range(KD):
                                mm = tensor.matmul(
                                    gq[:, sl:sl + 1, 0:NV],
                                    lhsT=ct16[:, k:k + 1, bt * P:(bt + 1) * P],
                                    rhs=wt_sb[k][:, g * GCOLS + nin * NV:
                                                 g * GCOLS + (nin + 1) * NV],
                                    start=(k == 0), stop=(k == KD - 1))
                        else:
                            tensor.matmul(
                                gq[:, sl:sl + 1, 0:NV],
                                lhsT=ct8[:, 0:2, mc * P:(mc + 1) * P],
                                rhs=wt8_sb[:, g * KD:g * KD + 2,
                                           nin * NV:(nin + 1) * NV],
                                start=True, stop=False, perf_mode=DR)
                            # rhs k2 plane repeated via stride-0 broadcast;
                            # the lhsT k3 plane is real zeros (DVE memset)
                            mm = tensor.matmul(
                                gq[:, sl:sl + 1, 0:NV],
                                lhsT=ct8[:, 2:4, mc * P:(mc + 1) * P],
                                rhs=wt8_sb[:, g * KD + 2:g * KD + 3,
                                           nin * NV:(nin + 1) * NV]
                                .broadcast_to([P, 2, NV]),
                                start=False, stop=True, perf_mode=DR)
                        mm.then_inc(pegemm, 1)

                # prefixes run in NATURAL chunk order (the carry chain),
                # two positions ahead of the block sweep
                prefix(0)
                prefix(1)
                for bi, (g, mc) in enumerate(block_seq()):
                    if g == 0:
                        j = bi // NGRP
                        if j + 2 < NCHT:
                            prefix(j + 2)
                    gemm_block(g, mc, bi)

        def make_evict(engine, mulop, mysem):
            def evict(it, bi, g, mc, pi):
                buf, sem, gi, nst = stg_buf(it, bi)
                if gi >= nst:
                    engine.wait_ge(sem, 16 * (gi // nst))
                a1 = it * GM_IT + bi * VGRP + pi * 2 + 2
                engine.wait_ge(pegemm, a1)
                a0 = it * GM_IT + bi * VGRP + pi * 2
                mulop(buf[:, pi * 2:(pi + 1) * 2, :],
                      gps[(a0 // 2) % 3][:, 0:2, 0:NV],
                      evc_ap(mc % NCHUNK)).then_inc(mysem, 1)
            return evict

        @blk.scalar
        def _(scalar):
            scalar.wait_ge(csem32, 16)
            evict = make_evict(scalar, scalar.mul, asem)
            for it in range(reps):
                for bi, (g, mc) in enumerate(block_seq()):
                    for pi in range(2):
                        if _pair_engine(2 * bi + pi) == "a":
                            evict(it, bi, g, mc, pi)

        @blk.vector
        def _(vector):
            # zero the padded 4th k-slice of NT: garbage fp8 here could be
            # NaN and poison the DoubleRow accumulation (the matching rhs
            # plane is a stride-0 repeat of real data, so lhsT zeros rule)
            vector.memset(ct8[:, 3:4, :], 0).then_inc(k3sem, 1)
            vector.wait_ge(csem32, 16)
            evict = make_evict(vector, vector.tensor_scalar_mul, dsem)
            for it in range(reps):
                def ntD(cc):
                    """Fused NT copy + carry update + next carrysc, on DVE."""
                    jc = it * NCHT + cc
                    c = cc % NCHUNK
                    if it > 0 and cc == 0:
                        # WAR: gemm of iter it-1 must be done reading NT
                        vector.wait_ge(pegemm, it * GM_IT)
                    vector.wait_ge(ctdone, jc * KD + KD)
                    src = ctps[jc % 2][:].rearrange("p (k t) -> p k t", k=KD)
                    if c == 0:
                        bt = cc // NCHUNK
                        vector.tensor_scalar_mul(
                            ct16[:, :, bt * P:(bt + 1) * P],
                            src, 1.0).then_inc(ctsbD, 1)
                    else:
                        cb = carrysc_sb[:, cc * KD:(cc + 1) * KD] \
                            .unsqueeze(-1).broadcast_to([P, KD, P])
                        vector.scalar_tensor_tensor(
                            ct8[:, 0:KD, cc * P:(cc + 1) * P], src,
                            float(2.0 ** -E_NT[c]), cb,
                            Alu.mult, Alu.add).then_inc(ctsbD, 1)
                    lastc = ctps[jc % 2][:, P - 1:KD * P:P]
                    dst = carry_sb[:, cc * KD:(cc + 1) * KD]
                    if c == 0:
                        vector.tensor_scalar_mul(dst, lastc, 1.0
                                                 ).then_inc(carrysem, 1)
                    else:
                        vector.tensor_tensor(
                            dst, lastc,
                            carry_sb[:, (cc - 1) * KD:cc * KD],
                            Alu.add).then_inc(carrysem, 1)
                    # pre-scaled carry for the NEXT chunk's fused NT (same
                    # engine, in order: no semaphore needed)
                    nxt = cc + 1
                    if nxt < NCHT and nxt % NCHUNK != 0:
                        vector.tensor_scalar_mul(
                            carrysc_sb[:, nxt * KD:(nxt + 1) * KD],
                            carry_sb[:, cc * KD:(cc + 1) * KD],
                            float(2.0 ** -E_NT[nxt % NCHUNK]))

                # NT chain in NATURAL chunk order, one position ahead
                ntD(0)
                ntD(1)
                for bi, (g, mc) in enumerate(block_seq()):
                    if g == 0:
                        j = bi // NGRP
                        if j + 2 < NCHT:
                            ntD(j + 2)
                    for pi in range(2):
                        if _pair_engine(2 * bi + pi) == "d":
                            evict(it, bi, g, mc, pi)

    return nc


@functools.lru_cache(maxsize=None)
def _get_program(has_bias: bool, reps: int = 1, dbg: bool = False):
    return _build(has_bias, reps, dbg)


@functools.lru_cache(maxsize=None)
def _host_consts():
    import ml_dtypes
    t = np.arange(T, dtype=np.float64)
    den = (t + 1.0) * (t + 2.0) / 2.0
    s = np.arange(P)
    tril_t = (s[:, None] <= s[None, :]).astype(np.float32)  # [s, t] s<=t
    c16 = np.zeros((P, NCHUNK * P), dtype=ml_dtypes.bfloat16)
    for c in range(NCHUNK):
        posw = (np.arange(c * P, (c + 1) * P, dtype=np.float32) + 1.0)
        c16[:, c * P:(c + 1) * P] = (posw[:, None] * tril_t
                                     ).astype(ml_dtypes.bfloat16)
    c32 = np.zeros((P, NCHUNK), dtype=np.float32)
    for c in range(NCHUNK):
        sc = 1.0 if c == 0 else 2.0 ** (E_NT[c] - EW + S_OUT[c])
        c32[:, c] = (sc / den[c * P:(c + 1) * P]).astype(np.float32)
    return c16, c32


def make_in_maps(context, emb_table, W, b):
    import ml_dtypes
    context = np.asarray(context)
    emb_table = np.asarray(emb_table, dtype=np.float32)
    W = np.asarray(W, dtype=np.float32)
    b = np.asarray(b, dtype=np.float32)
    has_bias = bool(np.any(b))

    table16 = np.ascontiguousarray(emb_table.astype(ml_dtypes.bfloat16))
    wt_full = np.ascontiguousarray(W.T.astype(ml_dtypes.bfloat16))   # (D, V)
    w8_full = (W.T.astype(np.float32) * (2.0 ** EW)).astype(
        ml_dtypes.float8_e4m3)                                       # (D, V)
    c16, c32 = _host_consts()

    in_maps = []
    for ci in range(NCORE):
        vg, bg = ci % NVG, ci // NVG
        idx = np.concatenate(
            [context[bg * NB + bt].reshape(NCHUNK, P).T for bt in range(NB)],
            axis=1).astype(np.int32)           # [p, cc]
        wt = np.ascontiguousarray(wt_full[:, vg * V_CORE:(vg + 1) * V_CORE])
        w8 = np.zeros((P, NGRP * KD, GCOLS), dtype=ml_dtypes.float8_e4m3)
        for g in range(NGRP):
            for k in range(KD):
                w8[:, g * KD + k, :] = w8_full[
                    k * P:(k + 1) * P,
                    vg * V_CORE + g * GCOLS:vg * V_CORE + (g + 1) * GCOLS]
        in_maps.append({"idx": np.ascontiguousarray(idx), "table": table16,
                        "wt": wt, "wt8": w8.reshape(P, NGRP * KD * GCOLS),
                        "cst16": c16, "cst32": c32})
    return in_maps, has_bias


def kernel(context, emb_table, W, b):
    in_maps, has_bias = make_in_maps(context, emb_table, W, b)
    nc = _get_program(False)
    res = None
    for attempt in range(3):
        try:
            res = run_bass_kernel_spmd(nc, in_maps, list(range(NCORE)))
            break
        except Exception:
            # transient NRT errors on the axon-tunneled device: retry
            if attempt == 2:
                raise
            import time
            time.sleep(10.0 * (attempt + 1))

    deq = np.array([2.0 ** -S_OUT[c] for c in range(1, NCHUNK)],
                   dtype=np.float32)[None, :, None, None]
    out = np.empty((B, T, V), dtype=np.float32)
    for ci in range(NCORE):
        vg, bg = ci % NVG, ci // NVG
        vsl = slice(vg * V_CORE, (vg + 1) * V_CORE)
        o16 = np.asarray(res.results[ci]["out16"]).astype(np.float32)
        o8 = (np.asarray(res.results[ci]["out8"]).astype(np.float32)
              .reshape(NB, NCHUNK - 1, P, V_CORE) * deq)
        for bt in range(NB):
            bfull = bg * NB + bt
            out[bfull, 0:P, vsl] = o16[bt * P:(bt + 1) * P]
            out[bfull, P:T, vsl] = o8[bt].reshape((NCHUNK - 1) * P, V_CORE)
    if has_bias:
        out += np.asarray(b, dtype=np.float32)[None, None, :]
    return out
